# revision 13
# baseline (speedup 1.0000x reference)
"""Trainium2 Bass kernel for nn_BilinearSentenceEncoder (v2).

Computes, for sentence [L=128, B=4096, D=300], size [B], W [D, D]:
  sym-pair scores s_{l}^T Wsym s_{l-1} (Wsym = (W+W.T)/2), self scores
  s_l^T Wsym s_l, 3-way masked softmax over (prev, self, next) channels,
  and the weighted combination out[l] = w1*s[l] + w0*s[l-1] + w2*s[l+1].

Sharding: data-parallel over B across 8 NeuronCores (512 batch columns
per core); W replicated.

v2 design (vs v0 baseline at 1.677 ms):
  - host ships TWO bf16 copies of s: L-major [L, BC, D] and D-major
    packed 'stp' [100, BC*3*128] (stp[d, (b,i,l)] = s[l,b,i*100+d]).
    This removes all PE transposes + their ACT drains.
  - Wsym is pre-scaled by 1/D and packed [100, 3*300] bf16.
  - per column b: V = (Wsym/D) s via 3 accumulating PE matmuls
    (lhsT = stp chunk, stationary; rhs = w).  V pairs drain to bf16
    SBUF via ACT.  DVE dots at bf16 2x: a = <V,s>, sym' = <V,s_down>
    (s_down = partition-shifted s via one SBUF-SBUF DMA per chunk;
    this replaces the baseline's second matmul burst).
  - masked 3-channel softmax on [128, CHUNK] tiles.
  - tridiagonal A^T built per column in bf16 (tensor_scalar 4x with
    fused e1*r + two STT accumulate ops), one combine matmul per
    column; out drains to bf16 and is stored as bf16 (host upcasts).
HBM per core: 2*39.3 MB in + 39.3 MB out (bf16) vs 78.6+78.6 f32.
"""

import sys

sys.path.insert(0, "/opt/trn_rl_repo")

import numpy as np
import ml_dtypes

import concourse.bacc as bacc
import concourse.mybir as mybir
from concourse import tile
from concourse.bass_utils import run_bass_kernel_spmd

dt = mybir.dt
AF = mybir.ActivationFunctionType
ALU = mybir.AluOpType

L, B, D = 128, 4096, 300
NCORES = 8
BC = B // NCORES          # 512 batch columns per core
CHUNK = 32                # batch columns per pipeline chunk
NCHUNK = BC // CHUNK      # 32
NEG = np.float32(-1.0e38)
DK = 100                  # contraction chunk (3 x 100 = 300)


def _build_nc():
    nc = bacc.Bacc()
    f32, bf16, f8 = dt.float32, dt.bfloat16, dt.float8e4

    s_in = nc.declare_dram_parameter("s", [L, BC, D], bf16, isOutput=False)
    s8_in = nc.declare_dram_parameter("s8", [L, BC, D], f8, isOutput=False)
    stp_in = nc.declare_dram_parameter("stp", [DK, BC * 384], f8, isOutput=False)
    w_in = nc.declare_dram_parameter("wsym", [DK, 3 * D], f8, isOutput=False)
    m0_in = nc.declare_dram_parameter("m0", [L, BC], f32, isOutput=False)
    m2_in = nc.declare_dram_parameter("m2", [L, BC], f32, isOutput=False)
    im_in = nc.declare_dram_parameter("imask", [128, 3 * 2048], bf16, isOutput=False)
    o_out = nc.declare_dram_parameter("o", [L, BC, D], bf16, isOutput=True)

    with tile.TileContext(nc) as tc:
        with (
            tc.tile_pool(name="const", bufs=1) as cpool,
            tc.tile_pool(name="s", bufs=3) as s_pool,
            tc.tile_pool(name="stp", bufs=3) as stp_pool,
            tc.tile_pool(name="sdn", bufs=2) as sdn_pool,
            tc.tile_pool(name="vs", bufs=4) as vs_pool,
            tc.tile_pool(name="scr", bufs=1) as scr_pool,
            tc.tile_pool(name="sc", bufs=2) as sc_pool,
            tc.tile_pool(name="atb", bufs=2) as atb_pool,
            tc.tile_pool(name="o", bufs=2) as o_pool,
            tc.tile_pool(name="vp", bufs=2, space="PSUM") as v_pool,
            tc.tile_pool(name="op", bufs=2, space="PSUM") as ops_pool,
        ):
            w_t = cpool.tile([DK, 3 * D], f8)
            im_t = cpool.tile([128, 3, 16, 128], bf16)
            tmp_t = cpool.tile([128, 2048], bf16, tag="at_tmp")
            m0_t = cpool.tile([L, BC], f32)
            m2_t = cpool.tile([L, BC], f32)
            nc.sync.dma_start(out=w_t[:, :], in_=w_in[:, :])
            nc.sync.dma_start(
                out=im_t[:, :, :, :],
                in_=im_in[:, :].rearrange("p (m j c) -> p m j c", m=3, j=16),
            )
            nc.sync.dma_start(out=m0_t[:, :], in_=m0_in[:, :])
            nc.sync.dma_start(out=m2_t[:, :], in_=m2_in[:, :])

            scr = scr_pool.tile([L, D], bf16, tag="scr_a")
            scr2 = scr_pool.tile([L, D], bf16, tag="scr_b")

            def emit_combine(prev):
                """Combine + drain + store for a finished chunk (emitted one
                iteration later so its matmuls sit behind the NEXT chunk's V
                matmuls in the PE FIFO and never stall it)."""
                atb_p, s_p, sl_p = prev
                o_t = o_pool.tile([L, CHUNK, D], bf16)
                for p in range(CHUNK // 2):
                    o_ps = ops_pool.tile([128, 2, 512], f32)
                    for jj in range(2):
                        j = 2 * p + jj
                        nc.tensor.matmul(
                            o_ps[:, jj, 0:D],
                            atb_p[:, j * 128 : (j + 1) * 128],
                            s_p[:, j, :],
                            start=True,
                            stop=True,
                        )
                    nc.scalar.activation(
                        o_t[:, 2 * p : 2 * p + 2, :], o_ps[:, :, 0:D], AF.Copy
                    )
                nc.scalar.dma_start(out=o_out[:, sl_p, :], in_=o_t[:, :, :])

            prev = None
            for c in range(NCHUNK):
                b0 = c * CHUNK
                sl = slice(b0, b0 + CHUNK)

                s_t = s_pool.tile([L, CHUNK, D], bf16)
                nc.sync.dma_start(out=s_t[:, :, :], in_=s_in[:, sl, :])
                stp_t = stp_pool.tile([DK, CHUNK * 384], f8)
                nc.sync.dma_start(
                    out=stp_t[:, :], in_=stp_in[:, b0 * 384 : (b0 + CHUNK) * 384]
                )
                # s_up[l] = s[l+1]: second HBM read of rows 1..127
                # (partition-shifted SBUF-SBUF copies shatter into
                # per-partition descriptors; an HBM slice read does not).
                # Partition 127 never read: sym dot runs on 0..126.
                sup_t = sdn_pool.tile([L, CHUNK, D], f8)
                nc.scalar.dma_start(out=sup_t[0:127, :, :], in_=s8_in[1:128, sl, :])

                a_t = sc_pool.tile([L, CHUNK], f32, tag="a_t")
                sym_t = sc_pool.tile([L, CHUNK], f32, tag="sym_t")
                nc.vector.memset(sym_t[:, :], 0.0)

                # ---- V matmuls + drains + dots, two columns at a time ----
                for p in range(CHUNK // 2):
                    v_ps = v_pool.tile([128, 2, 512], f32)
                    for jj in range(2):
                        j = 2 * p + jj
                        for i in range(3):
                            col0 = j * 384 + i * 128
                            nc.tensor.matmul(
                                v_ps[:, jj, 0:D],
                                stp_t[:, col0 : col0 + 128],
                                w_t[:, i * D : (i + 1) * D],
                                start=(i == 0),
                                stop=(i == 2),
                            )
                    vsb = vs_pool.tile([128, 2, D], bf16)
                    nc.scalar.activation(vsb[:, :, :], v_ps[:, :, 0:D], AF.Copy)
                    for jj in range(2):
                        j = 2 * p + jj
                        nc.vector.scalar_tensor_tensor(
                            out=scr[:, :],
                            in0=vsb[:, jj, :],
                            scalar=1.0 / D,
                            in1=s_t[:, j, :],
                            op0=ALU.mult,
                            op1=ALU.mult,
                            accum_out=a_t[:, j : j + 1],
                        )
                        nc.vector.scalar_tensor_tensor(
                            out=scr2[0:127, :],
                            in0=vsb[0:127, jj, :],
                            scalar=1.0 / D,
                            in1=sup_t[0:127, j, :],
                            op0=ALU.mult,
                            op1=ALU.mult,
                            accum_out=sym_t[0:127, j : j + 1],
                        )

                # previous chunk's combine goes to the PE queue here, AFTER
                # this chunk's V matmuls: its A^T inputs are already done,
                # so the PE never waits on this chunk's softmax.
                if prev is not None:
                    emit_combine(prev)

                # ---- chunk softmax over 3 channels ----
                # sym_t[l] = sym[l] (pair (l+1,l)); w2 logit[l] = sym[l]
                # (+m2); w0 logit[l] = sym[l-1] (+m0) via down-shift
                symdn = sc_pool.tile([L, CHUNK], f32, tag="symdn")
                nc.vector.memset(symdn[:, :], 0.0)
                nc.gpsimd.dma_start(out=symdn[1:128, :], in_=sym_t[0:127, :])

                l0_t = sc_pool.tile([L, CHUNK], f32, tag="l0")
                l2_t = sc_pool.tile([L, CHUNK], f32, tag="l2")
                nc.vector.tensor_tensor(
                    out=l0_t[:, :], in0=symdn[:, :], in1=m0_t[:, sl], op=ALU.add
                )
                nc.vector.tensor_tensor(
                    out=l2_t[:, :], in0=sym_t[:, :], in1=m2_t[:, sl], op=ALU.add
                )
                e0_t = sc_pool.tile([L, CHUNK], f32, tag="e0")
                e1_t = sc_pool.tile([L, CHUNK], f32, tag="e1")
                e2_t = sc_pool.tile([L, CHUNK], f32, tag="e2")
                nc.scalar.activation(e0_t[:, :], l0_t[:, :], AF.Exp)
                nc.scalar.activation(e1_t[:, :], a_t[:, :], AF.Exp)
                nc.scalar.activation(e2_t[:, :], l2_t[:, :], AF.Exp)
                den_t = sc_pool.tile([L, CHUNK], f32, tag="den")
                nc.vector.tensor_tensor(
                    out=den_t[:, :], in0=e0_t[:, :], in1=e1_t[:, :], op=ALU.add
                )
                nc.vector.tensor_tensor(
                    out=den_t[:, :], in0=den_t[:, :], in1=e2_t[:, :], op=ALU.add
                )
                r_t = sc_pool.tile([L, CHUNK], f32, tag="r")
                nc.vector.reciprocal(r_t[:, :], den_t[:, :])
                # softmax weights in bf16 (A^T operands); w0up[l] = w0[l+1],
                # w2dn[l] = w2[l-1] via SWDGE shifts that cast f32->bf16
                w1c = sc_pool.tile([L, CHUNK], bf16, tag="w1c")
                nc.vector.tensor_tensor(
                    out=w1c[:, :], in0=e1_t[:, :], in1=r_t[:, :], op=ALU.mult
                )
                w0c = sc_pool.tile([L, CHUNK], f32, tag="w0c")
                w2c = sc_pool.tile([L, CHUNK], f32, tag="w2c")
                nc.vector.tensor_tensor(
                    out=w0c[:, :], in0=e0_t[:, :], in1=r_t[:, :], op=ALU.mult
                )
                nc.vector.tensor_tensor(
                    out=w2c[:, :], in0=e2_t[:, :], in1=r_t[:, :], op=ALU.mult
                )
                w0up = sc_pool.tile([L, CHUNK], bf16, tag="w0up")
                w2dn = sc_pool.tile([L, CHUNK], bf16, tag="w2dn")
                nc.vector.memset(w0up[:, :], 0.0)
                nc.vector.memset(w2dn[:, :], 0.0)
                nc.gpsimd.dma_start(out=w0up[0:127, :], in_=w0c[1:128, :])
                nc.gpsimd.dma_start(out=w2dn[1:128, :], in_=w2c[0:127, :])

                # ---- tridiagonal A^T tiles (bf16), built 16 columns per
                # op: mask strips x broadcast per-column weights ----
                atb = atb_pool.tile([128, CHUNK * 128], bf16)
                for g in range(CHUNK // 16):
                    j0 = 16 * g
                    ag = atb[:, j0 * 128 : (j0 + 16) * 128].rearrange(
                        "p (j c) -> p j c", j=16
                    )
                    def wbc(t):
                        return t[:, j0 : j0 + 16].unsqueeze(2).broadcast_to(
                            [128, 16, 128]
                        )
                    nc.vector.tensor_tensor(
                        out=ag, in0=im_t[:, 0, :, :], in1=wbc(w1c), op=ALU.mult
                    )
                    tg = tmp_t[:, :].rearrange("p (j c) -> p j c", j=16)
                    ag2 = atb[:, j0 * 128 : (j0 + 16) * 128]
                    nc.vector.tensor_tensor(
                        out=tg, in0=im_t[:, 1, :, :], in1=wbc(w0up), op=ALU.mult
                    )
                    nc.vector.tensor_tensor(
                        out=ag2, in0=ag2, in1=tmp_t[:, :], op=ALU.add
                    )
                    nc.vector.tensor_tensor(
                        out=tg, in0=im_t[:, 2, :, :], in1=wbc(w2dn), op=ALU.mult
                    )
                    nc.vector.tensor_tensor(
                        out=ag2, in0=ag2, in1=tmp_t[:, :], op=ALU.add
                    )
                prev = (atb, s_t, sl)

            emit_combine(prev)

    nc.compile()
    return nc


_NC_CACHE = {}


def _get_nc():
    if "nc" not in _NC_CACHE:
        _NC_CACHE["nc"] = _build_nc()
    return _NC_CACHE["nc"]


def _host_inputs(sentence, size, W):
    sentence = np.asarray(sentence, dtype=np.float32)
    size = np.asarray(size).astype(np.int64)
    W = np.asarray(W, dtype=np.float32)

    bf = ml_dtypes.bfloat16
    f8 = ml_dtypes.float8_e4m3
    s_bf = sentence.astype(bf)                          # [L, B, D]
    s_f8 = sentence.astype(f8)
    # D-major packed transpose: stp[d, (b, i, l)] = s[l, b, i*100+d]
    srt = np.ascontiguousarray(
        s_f8.reshape(L, B, 3, DK).transpose(3, 1, 2, 0)
    )                                                   # [100, B, 3, 128]

    wsym = 0.5 * (W + W.T)                              # natural scale: fp8
    w_pack = np.zeros((DK, 3 * D), dtype=f8)            # e4m3 underflows at
    for i in range(3):                                  # Wsym/D scale; the
        w_pack[:, i * D : (i + 1) * D] = (              # 1/D rides the dot
            wsym[i * DK : (i + 1) * DK, :].astype(f8)   # STT scalar instead
        )

    I0 = np.eye(128, dtype=np.float32)
    Iup = np.zeros((128, 128), np.float32)
    Iup[np.arange(127), np.arange(1, 128)] = 1.0
    Idn = np.zeros((128, 128), np.float32)
    Idn[np.arange(1, 128), np.arange(127)] = 1.0
    imask = np.ascontiguousarray(
        np.concatenate(
            [np.tile(m, (1, 16)) for m in (I0, Iup, Idn)], axis=1
        ).astype(bf)
    )

    pos = np.arange(L, dtype=np.int64)[:, None]
    m0 = np.where(pos < size[None, :], 0.0, NEG).astype(np.float32)
    m0[0, :] = NEG
    m2 = np.where(pos < np.clip(size - 1, 0, None)[None, :], 0.0, NEG).astype(
        np.float32
    )
    m2[L - 1, :] = NEG

    in_maps = []
    for c in range(NCORES):
        bsl = slice(c * BC, (c + 1) * BC)
        in_maps.append(
            {
                "s": np.ascontiguousarray(s_bf[:, bsl, :]),
                "s8": np.ascontiguousarray(s_f8[:, bsl, :]),
                "stp": np.ascontiguousarray(srt[:, bsl].reshape(DK, BC * 384)),
                "wsym": w_pack,
                "m0": np.ascontiguousarray(m0[:, bsl]),
                "m2": np.ascontiguousarray(m2[:, bsl]),
                "imask": imask,
            }
        )
    return in_maps


def kernel(sentence, size, W):
    nc = _get_nc()
    in_maps = _host_inputs(sentence, size, W)
    res = run_bass_kernel_spmd(nc, in_maps, core_ids=list(range(NCORES)))
    out = np.concatenate([res.results[c]["o"] for c in range(NCORES)], axis=1)
    return out.astype(np.float32)


def _install_ntff_hook():
    """Register the axon NTFF profiling hook that this container's boot
    skipped (antenv.axon_hooks module absent)."""
    try:
        from antenv.axon_hooks import get_axon_ntff_profile_hook  # noqa: F401

        return
    except ImportError:
        pass
    import contextlib
    import ctypes
    import types

    so_path = "/opt/axon/libaxon_pjrt.so"
    lib = ctypes.CDLL(so_path)
    if not hasattr(lib, "axon_start_nrt_profile"):
        return
    lib.axon_start_nrt_profile.argtypes = [
        ctypes.POINTER(ctypes.c_int64),
        ctypes.c_size_t,
    ]
    lib.axon_start_nrt_profile.restype = ctypes.c_int64
    lib.axon_stop_nrt_profile.argtypes = [ctypes.c_char_p]
    lib.axon_stop_nrt_profile.restype = ctypes.c_int64

    @contextlib.contextmanager
    def _hook(output_dir, device_ids):
        import jax

        jax.devices()
        if device_ids:
            ids = (ctypes.c_int64 * len(device_ids))(*device_ids)
            rc = lib.axon_start_nrt_profile(ids, len(device_ids))
        else:
            rc = lib.axon_start_nrt_profile(None, 0)
        if rc != 0:
            raise RuntimeError(f"axon_start_nrt_profile rc={rc}")
        try:
            yield
        finally:
            n = lib.axon_stop_nrt_profile(str(output_dir).encode())
            print(f"ntff capture: {n} file(s) -> {output_dir}")

    mod = types.ModuleType("antenv.axon_hooks")
    mod.get_axon_ntff_profile_hook = lambda: _hook
    mod.set_axon_ntff_profile_hook = lambda h: None
    import antenv

    sys.modules["antenv.axon_hooks"] = mod
    antenv.axon_hooks = mod


def run_traced(sentence, size, W):
    """Like kernel(), but also returns (exec_time_ns, profile_json path)."""
    _install_ntff_hook()
    nc = _get_nc()
    in_maps = _host_inputs(sentence, size, W)
    res = run_bass_kernel_spmd(
        nc, in_maps, core_ids=list(range(NCORES)), trace=True, trace_cores=[0]
    )
    out = np.concatenate([res.results[c]["o"] for c in range(NCORES)], axis=1)
    return out.astype(np.float32), res.exec_time_ns, res.profile_json


if __name__ == "__main__":
    rng = np.random.default_rng(0)
    s = rng.standard_normal((L, B, D)).astype(np.float32)
    sz = rng.integers(0, L, size=(B,)).astype(np.int32)
    W = (rng.standard_normal((D, D)) / np.sqrt(D)).astype(np.float32)
    out = kernel(s, sz, W)
    print("out", out.shape, out.dtype, np.abs(out).max())


# revision 14
# speedup vs baseline: 1.4195x; 1.4195x over previous
"""Trainium2 Bass kernel for nn_BilinearSentenceEncoder (v2).

Computes, for sentence [L=128, B=4096, D=300], size [B], W [D, D]:
  sym-pair scores s_{l}^T Wsym s_{l-1} (Wsym = (W+W.T)/2), self scores
  s_l^T Wsym s_l, 3-way masked softmax over (prev, self, next) channels,
  and the weighted combination out[l] = w1*s[l] + w0*s[l-1] + w2*s[l+1].

Sharding: data-parallel over B across 8 NeuronCores (512 batch columns
per core); W replicated.

v2 design (vs v0 baseline at 1.677 ms):
  - host ships TWO bf16 copies of s: L-major [L, BC, D] and D-major
    packed 'stp' [100, BC*3*128] (stp[d, (b,i,l)] = s[l,b,i*100+d]).
    This removes all PE transposes + their ACT drains.
  - Wsym is pre-scaled by 1/D and packed [100, 3*300] bf16.
  - per column b: V = (Wsym/D) s via 3 accumulating PE matmuls
    (lhsT = stp chunk, stationary; rhs = w).  V pairs drain to bf16
    SBUF via ACT.  DVE dots at bf16 2x: a = <V,s>, sym' = <V,s_down>
    (s_down = partition-shifted s via one SBUF-SBUF DMA per chunk;
    this replaces the baseline's second matmul burst).
  - masked 3-channel softmax on [128, CHUNK] tiles.
  - tridiagonal A^T built per column in bf16 (tensor_scalar 4x with
    fused e1*r + two STT accumulate ops), one combine matmul per
    column; out drains to bf16 and is stored as bf16 (host upcasts).
HBM per core: 2*39.3 MB in + 39.3 MB out (bf16) vs 78.6+78.6 f32.
"""

import sys

sys.path.insert(0, "/opt/trn_rl_repo")

import numpy as np
import ml_dtypes

import concourse.bacc as bacc
import concourse.mybir as mybir
from concourse import tile
from concourse.bass_utils import run_bass_kernel_spmd

dt = mybir.dt
AF = mybir.ActivationFunctionType
ALU = mybir.AluOpType

L, B, D = 128, 4096, 300
NCORES = 8
BC = B // NCORES          # 512 batch columns per core
CHUNK = 32                # batch columns per pipeline chunk
NCHUNK = BC // CHUNK      # 32
NEG = np.float32(-1.0e38)
DK = 100                  # contraction chunk (3 x 100 = 300)


def _build_nc():
    nc = bacc.Bacc()
    f32, bf16, f8 = dt.float32, dt.bfloat16, dt.float8e4

    s_in = nc.declare_dram_parameter("s", [L, BC, D], bf16, isOutput=False)
    s8_in = nc.declare_dram_parameter("s8", [L, BC, D], f8, isOutput=False)
    stp_in = nc.declare_dram_parameter("stp", [DK, BC * 384], f8, isOutput=False)
    w_in = nc.declare_dram_parameter("wsym", [DK, 3 * D], f8, isOutput=False)
    m0_in = nc.declare_dram_parameter("m0", [L, BC], f32, isOutput=False)
    m2_in = nc.declare_dram_parameter("m2", [L, BC], f32, isOutput=False)
    im_in = nc.declare_dram_parameter("imask", [128, 3 * 2048], bf16, isOutput=False)
    o_out = nc.declare_dram_parameter("o", [L, BC, D], bf16, isOutput=True)

    with tile.TileContext(nc) as tc:
        with (
            tc.tile_pool(name="const", bufs=1) as cpool,
            tc.tile_pool(name="s", bufs=3) as s_pool,
            tc.tile_pool(name="stp", bufs=3) as stp_pool,
            tc.tile_pool(name="sdn", bufs=2) as sdn_pool,
            tc.tile_pool(name="vs", bufs=4) as vs_pool,
            tc.tile_pool(name="scr", bufs=1) as scr_pool,
            tc.tile_pool(name="sc", bufs=2) as sc_pool,
            tc.tile_pool(name="atb", bufs=2) as atb_pool,
            tc.tile_pool(name="o", bufs=2) as o_pool,
            tc.tile_pool(name="vp", bufs=2, space="PSUM") as v_pool,
            tc.tile_pool(name="op", bufs=2, space="PSUM") as ops_pool,
        ):
            w_t = cpool.tile([DK, 3 * D], f8)
            im_t = cpool.tile([128, 3, 16, 128], bf16)
            tmp_t = cpool.tile([128, 2048], bf16, tag="at_tmp")
            m0_t = cpool.tile([L, BC], f32)
            m2_t = cpool.tile([L, BC], f32)
            nc.sync.dma_start(out=w_t[:, :], in_=w_in[:, :])
            nc.sync.dma_start(
                out=im_t[:, :, :, :],
                in_=im_in[:, :].rearrange("p (m j c) -> p m j c", m=3, j=16),
            )
            nc.sync.dma_start(out=m0_t[:, :], in_=m0_in[:, :])
            nc.sync.dma_start(out=m2_t[:, :], in_=m2_in[:, :])

            scr = scr_pool.tile([L, D], bf16, tag="scr_a")
            scr2 = scr_pool.tile([L, D], bf16, tag="scr_b")

            def emit_combine(prev):
                """Combine + drain + store for a finished chunk (emitted one
                iteration later so its matmuls sit behind the NEXT chunk's V
                matmuls in the PE FIFO and never stall it)."""
                atb_p, s_p, sl_p = prev
                o_t = o_pool.tile([L, CHUNK, D], bf16)
                for p in range(CHUNK // 2):
                    o_ps = ops_pool.tile([128, 2, 512], f32)
                    for jj in range(2):
                        j = 2 * p + jj
                        nc.tensor.matmul(
                            o_ps[:, jj, 0:D],
                            atb_p[:, j * 128 : (j + 1) * 128],
                            s_p[:, j, :],
                            start=True,
                            stop=True,
                        )
                    nc.scalar.activation(
                        o_t[:, 2 * p : 2 * p + 2, :], o_ps[:, :, 0:D], AF.Copy
                    )
                nc.scalar.dma_start(out=o_out[:, sl_p, :], in_=o_t[:, :, :])

            prev = None
            for c in range(NCHUNK):
                b0 = c * CHUNK
                sl = slice(b0, b0 + CHUNK)

                s_t = s_pool.tile([L, CHUNK, D], bf16)
                nc.sync.dma_start(out=s_t[:, :, :], in_=s_in[:, sl, :])
                stp_t = stp_pool.tile([DK, CHUNK * 384], f8)
                nc.sync.dma_start(
                    out=stp_t[:, :], in_=stp_in[:, b0 * 384 : (b0 + CHUNK) * 384]
                )
                # s_up[l] = s[l+1]: second HBM read of rows 1..127
                # (partition-shifted SBUF-SBUF copies shatter into
                # per-partition descriptors; an HBM slice read does not).
                # Partition 127 never read: sym dot runs on 0..126.
                sup_t = sdn_pool.tile([L, CHUNK, D], f8)
                nc.scalar.dma_start(out=sup_t[0:127, :, :], in_=s8_in[1:128, sl, :])

                a_t = sc_pool.tile([L, CHUNK], f32, tag="a_t")
                sym_t = sc_pool.tile([L, CHUNK], f32, tag="sym_t")
                nc.vector.memset(sym_t[:, :], 0.0)

                # ---- V matmuls + drains + dots, two columns at a time ----
                for p in range(CHUNK // 2):
                    v_ps = v_pool.tile([128, 2, 512], f32)
                    for jj in range(2):
                        j = 2 * p + jj
                        for i in range(3):
                            col0 = j * 384 + i * 128
                            nc.tensor.matmul(
                                v_ps[:, jj, 0:D],
                                stp_t[:, col0 : col0 + 128],
                                w_t[:, i * D : (i + 1) * D],
                                start=(i == 0),
                                stop=(i == 2),
                            )
                    vsb = vs_pool.tile([128, 2, D], bf16)
                    nc.scalar.activation(vsb[:, :, :], v_ps[:, :, 0:D], AF.Copy)
                    for jj in range(2):
                        j = 2 * p + jj
                        nc.vector.scalar_tensor_tensor(
                            out=scr[:, :],
                            in0=vsb[:, jj, :],
                            scalar=1.0 / D,
                            in1=s_t[:, j, :],
                            op0=ALU.mult,
                            op1=ALU.mult,
                            accum_out=a_t[:, j : j + 1],
                        )
                        nc.vector.scalar_tensor_tensor(
                            out=scr2[0:127, :],
                            in0=vsb[0:127, jj, :],
                            scalar=1.0 / D,
                            in1=sup_t[0:127, j, :],
                            op0=ALU.mult,
                            op1=ALU.mult,
                            accum_out=sym_t[0:127, j : j + 1],
                        )

                # previous chunk's combine goes to the PE queue here, AFTER
                # this chunk's V matmuls: its A^T inputs are already done,
                # so the PE never waits on this chunk's softmax.
                if prev is not None:
                    emit_combine(prev)

                # ---- chunk softmax over 3 channels ----
                # sym_t[l] = sym[l] (pair (l+1,l)); w2 logit[l] = sym[l]
                # (+m2); w0 logit[l] = sym[l-1] (+m0) via down-shift
                symdn = sc_pool.tile([L, CHUNK], f32, tag="symdn")
                nc.vector.memset(symdn[:, :], 0.0)
                nc.gpsimd.dma_start(out=symdn[1:128, :], in_=sym_t[0:127, :])

                l0_t = sc_pool.tile([L, CHUNK], f32, tag="l0")
                l2_t = sc_pool.tile([L, CHUNK], f32, tag="l2")
                nc.vector.tensor_tensor(
                    out=l0_t[:, :], in0=symdn[:, :], in1=m0_t[:, sl], op=ALU.add
                )
                nc.vector.tensor_tensor(
                    out=l2_t[:, :], in0=sym_t[:, :], in1=m2_t[:, sl], op=ALU.add
                )
                e0_t = sc_pool.tile([L, CHUNK], f32, tag="e0")
                e1_t = sc_pool.tile([L, CHUNK], f32, tag="e1")
                e2_t = sc_pool.tile([L, CHUNK], f32, tag="e2")
                nc.scalar.activation(e0_t[:, :], l0_t[:, :], AF.Exp)
                nc.scalar.activation(e1_t[:, :], a_t[:, :], AF.Exp)
                nc.scalar.activation(e2_t[:, :], l2_t[:, :], AF.Exp)
                den_t = sc_pool.tile([L, CHUNK], f32, tag="den")
                nc.vector.tensor_tensor(
                    out=den_t[:, :], in0=e0_t[:, :], in1=e1_t[:, :], op=ALU.add
                )
                nc.vector.tensor_tensor(
                    out=den_t[:, :], in0=den_t[:, :], in1=e2_t[:, :], op=ALU.add
                )
                r_t = sc_pool.tile([L, CHUNK], f32, tag="r")
                nc.vector.reciprocal(r_t[:, :], den_t[:, :])
                # softmax weights in bf16 (A^T operands); w0up[l] = w0[l+1],
                # w2dn[l] = w2[l-1] via SWDGE shifts that cast f32->bf16
                w1c = sc_pool.tile([L, CHUNK], f32, tag="w1c")
                nc.vector.tensor_tensor(
                    out=w1c[:, :], in0=e1_t[:, :], in1=r_t[:, :], op=ALU.mult
                )
                w0c = sc_pool.tile([L, CHUNK], f32, tag="w0c")
                w2c = sc_pool.tile([L, CHUNK], f32, tag="w2c")
                nc.vector.tensor_tensor(
                    out=w0c[:, :], in0=e0_t[:, :], in1=r_t[:, :], op=ALU.mult
                )
                nc.vector.tensor_tensor(
                    out=w2c[:, :], in0=e2_t[:, :], in1=r_t[:, :], op=ALU.mult
                )
                w0up = sc_pool.tile([L, CHUNK], f32, tag="w0up")
                w2dn = sc_pool.tile([L, CHUNK], f32, tag="w2dn")
                nc.vector.memset(w0up[:, :], 0.0)
                nc.vector.memset(w2dn[:, :], 0.0)
                nc.gpsimd.dma_start(out=w0up[0:127, :], in_=w0c[1:128, :])
                nc.gpsimd.dma_start(out=w2dn[1:128, :], in_=w2c[0:127, :])

                # ---- tridiagonal A^T tiles (bf16), built 16 columns per
                # op: mask strips x broadcast per-column weights ----
                atb = atb_pool.tile([128, CHUNK * 128], bf16)
                for g in range(CHUNK // 16):
                    j0 = 16 * g
                    ag = atb[:, j0 * 128 : (j0 + 16) * 128].rearrange(
                        "p (j c) -> p j c", j=16
                    )
                    def wbc(t):
                        return t[:, j0 : j0 + 16].unsqueeze(2).broadcast_to(
                            [128, 16, 128]
                        )
                    nc.vector.tensor_tensor(
                        out=ag, in0=im_t[:, 0, :, :], in1=wbc(w1c), op=ALU.mult
                    )
                    tg = tmp_t[:, :].rearrange("p (j c) -> p j c", j=16)
                    ag2 = atb[:, j0 * 128 : (j0 + 16) * 128]
                    nc.vector.tensor_tensor(
                        out=tg, in0=im_t[:, 1, :, :], in1=wbc(w0up), op=ALU.mult
                    )
                    nc.vector.tensor_tensor(
                        out=ag2, in0=ag2, in1=tmp_t[:, :], op=ALU.add
                    )
                    nc.vector.tensor_tensor(
                        out=tg, in0=im_t[:, 2, :, :], in1=wbc(w2dn), op=ALU.mult
                    )
                    nc.vector.tensor_tensor(
                        out=ag2, in0=ag2, in1=tmp_t[:, :], op=ALU.add
                    )
                prev = (atb, s_t, sl)

            emit_combine(prev)

    nc.compile()
    return nc


_NC_CACHE = {}


def _get_nc():
    if "nc" not in _NC_CACHE:
        _NC_CACHE["nc"] = _build_nc()
    return _NC_CACHE["nc"]


def _host_inputs(sentence, size, W):
    sentence = np.asarray(sentence, dtype=np.float32)
    size = np.asarray(size).astype(np.int64)
    W = np.asarray(W, dtype=np.float32)

    bf = ml_dtypes.bfloat16
    f8 = ml_dtypes.float8_e4m3
    s_bf = sentence.astype(bf)                          # [L, B, D]
    s_f8 = sentence.astype(f8)
    # D-major packed transpose: stp[d, (b, i, l)] = s[l, b, i*100+d]
    srt = np.ascontiguousarray(
        s_f8.reshape(L, B, 3, DK).transpose(3, 1, 2, 0)
    )                                                   # [100, B, 3, 128]

    wsym = 0.5 * (W + W.T)                              # natural scale: fp8
    w_pack = np.zeros((DK, 3 * D), dtype=f8)            # e4m3 underflows at
    for i in range(3):                                  # Wsym/D scale; the
        w_pack[:, i * D : (i + 1) * D] = (              # 1/D rides the dot
            wsym[i * DK : (i + 1) * DK, :].astype(f8)   # STT scalar instead
        )

    I0 = np.eye(128, dtype=np.float32)
    Iup = np.zeros((128, 128), np.float32)
    Iup[np.arange(127), np.arange(1, 128)] = 1.0
    Idn = np.zeros((128, 128), np.float32)
    Idn[np.arange(1, 128), np.arange(127)] = 1.0
    imask = np.ascontiguousarray(
        np.concatenate(
            [np.tile(m, (1, 16)) for m in (I0, Iup, Idn)], axis=1
        ).astype(bf)
    )

    pos = np.arange(L, dtype=np.int64)[:, None]
    m0 = np.where(pos < size[None, :], 0.0, NEG).astype(np.float32)
    m0[0, :] = NEG
    m2 = np.where(pos < np.clip(size - 1, 0, None)[None, :], 0.0, NEG).astype(
        np.float32
    )
    m2[L - 1, :] = NEG

    in_maps = []
    for c in range(NCORES):
        bsl = slice(c * BC, (c + 1) * BC)
        in_maps.append(
            {
                "s": np.ascontiguousarray(s_bf[:, bsl, :]),
                "s8": np.ascontiguousarray(s_f8[:, bsl, :]),
                "stp": np.ascontiguousarray(srt[:, bsl].reshape(DK, BC * 384)),
                "wsym": w_pack,
                "m0": np.ascontiguousarray(m0[:, bsl]),
                "m2": np.ascontiguousarray(m2[:, bsl]),
                "imask": imask,
            }
        )
    return in_maps


def kernel(sentence, size, W):
    nc = _get_nc()
    in_maps = _host_inputs(sentence, size, W)
    res = run_bass_kernel_spmd(nc, in_maps, core_ids=list(range(NCORES)))
    out = np.concatenate([res.results[c]["o"] for c in range(NCORES)], axis=1)
    return out.astype(np.float32)


def _install_ntff_hook():
    """Register the axon NTFF profiling hook that this container's boot
    skipped (antenv.axon_hooks module absent)."""
    try:
        from antenv.axon_hooks import get_axon_ntff_profile_hook  # noqa: F401

        return
    except ImportError:
        pass
    import contextlib
    import ctypes
    import types

    so_path = "/opt/axon/libaxon_pjrt.so"
    lib = ctypes.CDLL(so_path)
    if not hasattr(lib, "axon_start_nrt_profile"):
        return
    lib.axon_start_nrt_profile.argtypes = [
        ctypes.POINTER(ctypes.c_int64),
        ctypes.c_size_t,
    ]
    lib.axon_start_nrt_profile.restype = ctypes.c_int64
    lib.axon_stop_nrt_profile.argtypes = [ctypes.c_char_p]
    lib.axon_stop_nrt_profile.restype = ctypes.c_int64

    @contextlib.contextmanager
    def _hook(output_dir, device_ids):
        import jax

        jax.devices()
        if device_ids:
            ids = (ctypes.c_int64 * len(device_ids))(*device_ids)
            rc = lib.axon_start_nrt_profile(ids, len(device_ids))
        else:
            rc = lib.axon_start_nrt_profile(None, 0)
        if rc != 0:
            raise RuntimeError(f"axon_start_nrt_profile rc={rc}")
        try:
            yield
        finally:
            n = lib.axon_stop_nrt_profile(str(output_dir).encode())
            print(f"ntff capture: {n} file(s) -> {output_dir}")

    mod = types.ModuleType("antenv.axon_hooks")
    mod.get_axon_ntff_profile_hook = lambda: _hook
    mod.set_axon_ntff_profile_hook = lambda h: None
    import antenv

    sys.modules["antenv.axon_hooks"] = mod
    antenv.axon_hooks = mod


def run_traced(sentence, size, W):
    """Like kernel(), but also returns (exec_time_ns, profile_json path)."""
    _install_ntff_hook()
    nc = _get_nc()
    in_maps = _host_inputs(sentence, size, W)
    res = run_bass_kernel_spmd(
        nc, in_maps, core_ids=list(range(NCORES)), trace=True, trace_cores=[0]
    )
    out = np.concatenate([res.results[c]["o"] for c in range(NCORES)], axis=1)
    return out.astype(np.float32), res.exec_time_ns, res.profile_json


if __name__ == "__main__":
    rng = np.random.default_rng(0)
    s = rng.standard_normal((L, B, D)).astype(np.float32)
    sz = rng.integers(0, L, size=(B,)).astype(np.int32)
    W = (rng.standard_normal((D, D)) / np.sqrt(D)).astype(np.float32)
    out = kernel(s, sz, W)
    print("out", out.shape, out.dtype, np.abs(out).max())
